# revision 15
# baseline (speedup 1.0000x reference)
"""BinaryMatchAttention Trainium2 kernel.

reference semantics (per batch b):
    qb[k]   = (query_addr >> k) & 1                 k in [0, 16)
    w[s]    = prod_k (1 - |x[b, s, 96+k] - qb[k]|)
    out[b,d]= sum_s w[s] * x[b, s, d]               d in [0, 96)

Sharding: data-parallel over batch, one NeuronCore per batch element
(B == 8 == n_cores), no collectives.

Per-core plan (x_core [32768, 128] fp32 in HBM):
  - seq is chunked into 256 chunks of 128 positions; partition dim = seq%128.
  - NB=4 outer iterations each load T=64 chunks as one [128, 64, 112] SBUF
    tile (cols 0:112 only; cols 112:128 are never used).
  - match weights on DVE/ACT: d = bits - qb (DVE), a = |d| (ACT),
    m = 1 - a (DVE 2x tensor_scalar), then 4 strided pairwise products
    (DVE) reduce 16 -> 1 giving w [128, T].
  - einsum on TensorE: for each group of C chunks,
    psum[C, C*96] += w[:, Cj:Cj+C].T @ v[:, Cj:Cj+C, 0:96],
    accumulated across all chunk groups in one PSUM bank. Only the
    diagonal 96-blocks are wanted; host extracts + sums them.
"""

import os
import sys

if "/opt/trn_rl_repo" not in sys.path:
    sys.path.insert(0, "/opt/trn_rl_repo")

import numpy as np

S, D = 32768, 128
VD = 96          # value payload dims
NBITS = 16
BIT0 = 96
P = 128          # partitions = seq positions per chunk
# Wave sizes (in 128-seq chunks): T=32 balances DVE per-op overhead
# against pipeline granularity. The ramp at the start primes the
# DMA-descriptor pipeline quickly; the taper at the end shrinks the
# serial tail after the last DMA lands.
TS = [8, 16] + [32] * 7 + [4, 4]
assert sum(TS) == S // P
TMAX = max(TS)
NCORES = 8

# "f32"  : plain fp32 matmuls, C=1 (4 cycles/row on PE)
# "f32r" : float32r matmuls, C=4 (1 cycle/row when N>=256)
MM_MODE = os.environ.get("BMA_MM_MODE", "f32")

_CACHE = {}


def _chunks_per_mm(mode):
    return 4 if mode == "f32r" else 1


def _build(mode):
    import concourse.bacc as bacc
    import concourse.mybir as mybir
    import concourse.tile as tile

    f32 = mybir.dt.float32
    f32r = mybir.dt.float32r
    C = _chunks_per_mm(mode)

    nc = bacc.Bacc("TRN2", target_bir_lowering=False, debug=False)
    # For f32r mode the DRAM input is declared float32r (same bits as
    # float32) so the value tile arrives in SBUF already typed f32r.
    x_dt = f32r if mode == "f32r" else f32
    x = nc.dram_tensor("x", [S, D], x_dt, kind="ExternalInput")
    cq = nc.dram_tensor("cq", [P, NBITS], f32, kind="ExternalInput")
    out = nc.dram_tensor("out", [C, C * VD], f32, kind="ExternalOutput")

    # [128(part), 256(chunk), 128(col)]
    xc = x.ap().rearrange("(c p) d -> p c d", p=P)

    total_groups = sum(t // C for t in TS)
    last_g = total_groups - 1

    with tile.TileContext(nc) as tc:
        with (
            tc.tile_pool(name="const", bufs=1) as cpool,
            tc.tile_pool(name="v", bufs=5) as vpool,
            tc.tile_pool(name="wk", bufs=2) as wpool,
            tc.tile_pool(name="ps", bufs=1, space="PSUM") as ppool,
            tc.tile_pool(name="res", bufs=1) as rpool,
        ):
            cqt = cpool.tile([P, 1, NBITS], f32)
            nc.sync.dma_start(cqt[:], cq.ap().rearrange("p (o k) -> p o k", o=1))

            acc = ppool.tile([C, C * VD], f32)

            g = 0
            c0 = 0
            for ib, T in enumerate(TS):
                vt = vpool.tile([P, T, 112], x_dt, tag="vt")
                # All value loads go through Sync: its only job is to keep
                # descriptors streaming into the HWDGE ring. ACT stays free
                # for the Abs step so the weight chain is never parked
                # behind a blocking descriptor push.
                nc.sync.dma_start(vt[:], xc[:, c0 : c0 + T, 0:112])
                c0 += T

                bits = vt[:, :, BIT0 : BIT0 + NBITS]
                if mode == "f32r":
                    bits = bits.bitcast(f32)
                d = wpool.tile([P, T, NBITS], f32, tag="d")
                nc.vector.tensor_sub(d[:], bits, cqt[:].broadcast_to([P, T, NBITS]))
                # na = min(-d, d) = -|d|; avoids the ACT engine entirely
                # (its activation-table preamble load gates kernel start)
                na = wpool.tile([P, T, NBITS], f32, tag="na")
                nc.vector.scalar_tensor_tensor(
                    na[:], d[:], -1.0, d[:],
                    op0=mybir.AluOpType.mult, op1=mybir.AluOpType.min,
                )
                m = wpool.tile([P, T, NBITS], f32, tag="m")
                nc.vector.tensor_scalar(
                    m[:], na[:], 1.0, None, op0=mybir.AluOpType.add,
                )
                p8 = wpool.tile([P, T, 8], f32, tag="p8")
                nc.vector.tensor_mul(p8[:], m[:, :, 0::2], m[:, :, 1::2])
                p4 = wpool.tile([P, T, 4], f32, tag="p4")
                nc.vector.tensor_mul(p4[:], p8[:, :, 0::2], p8[:, :, 1::2])
                p2 = wpool.tile([P, T, 2], f32, tag="p2")
                nc.vector.tensor_mul(p2[:], p4[:, :, 0::2], p4[:, :, 1::2])
                # final tree level writes the weight tile, rounded to the
                # matmul dtype so the verifier sees an f32r producer
                w = wpool.tile([P, T, 1], x_dt, tag="w")
                nc.vector.tensor_mul(w[:], p2[:, :, 0::2], p2[:, :, 1::2])

                for j in range(T // C):
                    lhsT = w[:, j * C : (j + 1) * C, 0]        # [128, C]
                    rhs = vt[:, j * C : (j + 1) * C, 0:VD]     # [128, C, 96]
                    nc.tensor.matmul(
                        acc[:],
                        lhsT,
                        rhs,
                        start=(g == 0),
                        stop=(g == last_g),
                    )
                    g += 1

            res = rpool.tile([C, C * VD], f32)
            nc.vector.tensor_copy(res[:], acc[:])
            nc.sync.dma_start(out.ap(), res[:])

    nc.compile()
    return nc


def _get_nc(mode):
    if mode not in _CACHE:
        _CACHE[mode] = _build(mode)
    return _CACHE[mode]


def run(x, query_addr, trace=False, mode=None):
    """Returns (output [B, 96] float32, BassKernelResults)."""
    from concourse.bass_utils import run_bass_kernel_spmd

    mode = mode or MM_MODE
    C = _chunks_per_mm(mode)
    x = np.asarray(x)
    qa = int(np.asarray(query_addr))
    assert x.shape == (NCORES, S, D), x.shape

    qb = np.array([(qa >> k) & 1 for k in range(NBITS)], dtype=np.float32)
    cq = np.ascontiguousarray(np.broadcast_to(qb, (P, NBITS)))

    nc = _get_nc(mode)
    in_maps = [
        {"x": np.ascontiguousarray(x[b], dtype=np.float32), "cq": cq}
        for b in range(NCORES)
    ]
    kres = run_bass_kernel_spmd(nc, in_maps, list(range(NCORES)), trace=trace)

    outs = []
    for r in kres.results:
        o = np.asarray(r["out"]).reshape(C, C, VD)
        outs.append(o[np.arange(C), np.arange(C)].sum(axis=0))
    return np.stack(outs).astype(np.float32), kres


def kernel(x, query_addr):
    return run(x, query_addr)[0]


# revision 17
# speedup vs baseline: 1.1290x; 1.1290x over previous
"""BinaryMatchAttention Trainium2 kernel.

reference semantics (per batch b):
    qb[k]   = (query_addr >> k) & 1                 k in [0, 16)
    w[s]    = prod_k (1 - |x[b, s, 96+k] - qb[k]|)
    out[b,d]= sum_s w[s] * x[b, s, d]               d in [0, 96)

Sharding: data-parallel over batch, one NeuronCore per batch element
(B == 8 == n_cores), no collectives.

Per-core plan (x_core [32768, 128] fp32 in HBM):
  - seq is chunked into 256 chunks of 128 positions; partition dim = seq%128.
  - NB=4 outer iterations each load T=64 chunks as one [128, 64, 112] SBUF
    tile (cols 0:112 only; cols 112:128 are never used).
  - match weights on DVE/ACT: d = bits - qb (DVE), a = |d| (ACT),
    m = 1 - a (DVE 2x tensor_scalar), then 4 strided pairwise products
    (DVE) reduce 16 -> 1 giving w [128, T].
  - einsum on TensorE: for each group of C chunks,
    psum[C, C*96] += w[:, Cj:Cj+C].T @ v[:, Cj:Cj+C, 0:96],
    accumulated across all chunk groups in one PSUM bank. Only the
    diagonal 96-blocks are wanted; host extracts + sums them.
"""

import os
import sys

if "/opt/trn_rl_repo" not in sys.path:
    sys.path.insert(0, "/opt/trn_rl_repo")

import numpy as np

S, D = 32768, 128
VD = 96          # value payload dims
NBITS = 16
BIT0 = 96
P = 128          # partitions = seq positions per chunk
# Wave sizes (in 128-seq chunks): T=32 balances DVE per-op overhead
# against pipeline granularity. The ramp at the start primes the
# DMA-descriptor pipeline quickly; the taper at the end shrinks the
# serial tail after the last DMA lands.
TS = [32] * 7 + [16, 8, 4, 4]
assert sum(TS) == S // P
TMAX = max(TS)
NCORES = 8

# "f32"  : plain fp32 matmuls, C=1 (4 cycles/row on PE)
# "f32r" : float32r matmuls, C=4 (1 cycle/row when N>=256)
MM_MODE = os.environ.get("BMA_MM_MODE", "f32")

_CACHE = {}


def _chunks_per_mm(mode):
    return 4 if mode == "f32r" else 1


def _build(mode):
    import concourse.bacc as bacc
    import concourse.mybir as mybir
    import concourse.tile as tile

    f32 = mybir.dt.float32
    f32r = mybir.dt.float32r
    C = _chunks_per_mm(mode)

    nc = bacc.Bacc("TRN2", target_bir_lowering=False, debug=False)
    # For f32r mode the DRAM input is declared float32r (same bits as
    # float32) so the value tile arrives in SBUF already typed f32r.
    x_dt = f32r if mode == "f32r" else f32
    x = nc.dram_tensor("x", [S, D], x_dt, kind="ExternalInput")
    cq = nc.dram_tensor("cq", [P, NBITS], f32, kind="ExternalInput")
    out = nc.dram_tensor("out", [C, C * VD], f32, kind="ExternalOutput")

    # [128(part), 256(chunk), 128(col)]
    xc = x.ap().rearrange("(c p) d -> p c d", p=P)

    total_groups = sum(t // C for t in TS)
    last_g = total_groups - 1

    with tile.TileContext(nc) as tc:
        with (
            tc.tile_pool(name="const", bufs=1) as cpool,
            tc.tile_pool(name="v", bufs=5) as vpool,
            tc.tile_pool(name="wk", bufs=2) as wpool,
            tc.tile_pool(name="ps", bufs=1, space="PSUM") as ppool,
            tc.tile_pool(name="res", bufs=1) as rpool,
        ):
            cqt = cpool.tile([P, 1, NBITS], f32)
            nc.sync.dma_start(cqt[:], cq.ap().rearrange("p (o k) -> p o k", o=1))

            acc = ppool.tile([C, C * VD], f32)

            g = 0
            c0 = 0
            for ib, T in enumerate(TS):
                vt = vpool.tile([P, T, 112], x_dt, tag="vt")
                # All value loads go through Sync: its only job is to keep
                # descriptors streaming into the HWDGE ring. ACT stays free
                # for the Abs step so the weight chain is never parked
                # behind a blocking descriptor push.
                nc.sync.dma_start(vt[:], xc[:, c0 : c0 + T, 0:112])
                c0 += T

                bits = vt[:, :, BIT0 : BIT0 + NBITS]
                if mode == "f32r":
                    bits = bits.bitcast(f32)
                d = wpool.tile([P, T, NBITS], f32, tag="d")
                nc.vector.tensor_sub(d[:], bits, cqt[:].broadcast_to([P, T, NBITS]))
                a = wpool.tile([P, T, NBITS], f32, tag="a")
                nc.scalar.activation(a[:], d[:], mybir.ActivationFunctionType.Abs)
                m = wpool.tile([P, T, NBITS], f32, tag="m")
                nc.vector.tensor_scalar(
                    m[:], a[:], -1.0, 1.0,
                    op0=mybir.AluOpType.mult, op1=mybir.AluOpType.add,
                )
                p8 = wpool.tile([P, T, 8], f32, tag="p8")
                nc.vector.tensor_mul(p8[:], m[:, :, 0::2], m[:, :, 1::2])
                p4 = wpool.tile([P, T, 4], f32, tag="p4")
                nc.vector.tensor_mul(p4[:], p8[:, :, 0::2], p8[:, :, 1::2])
                p2 = wpool.tile([P, T, 2], f32, tag="p2")
                nc.vector.tensor_mul(p2[:], p4[:, :, 0::2], p4[:, :, 1::2])
                # final tree level writes the weight tile, rounded to the
                # matmul dtype so the verifier sees an f32r producer
                w = wpool.tile([P, T, 1], x_dt, tag="w")
                nc.vector.tensor_mul(w[:], p2[:, :, 0::2], p2[:, :, 1::2])

                for j in range(T // C):
                    lhsT = w[:, j * C : (j + 1) * C, 0]        # [128, C]
                    rhs = vt[:, j * C : (j + 1) * C, 0:VD]     # [128, C, 96]
                    nc.tensor.matmul(
                        acc[:],
                        lhsT,
                        rhs,
                        start=(g == 0),
                        stop=(g == last_g),
                    )
                    g += 1

            res = rpool.tile([C, C * VD], f32)
            nc.vector.tensor_copy(res[:], acc[:])
            nc.sync.dma_start(out.ap(), res[:])

    nc.compile()
    return nc


def _get_nc(mode):
    if mode not in _CACHE:
        _CACHE[mode] = _build(mode)
    return _CACHE[mode]


def run(x, query_addr, trace=False, mode=None):
    """Returns (output [B, 96] float32, BassKernelResults)."""
    from concourse.bass_utils import run_bass_kernel_spmd

    mode = mode or MM_MODE
    C = _chunks_per_mm(mode)
    x = np.asarray(x)
    qa = int(np.asarray(query_addr))
    assert x.shape == (NCORES, S, D), x.shape

    qb = np.array([(qa >> k) & 1 for k in range(NBITS)], dtype=np.float32)
    cq = np.ascontiguousarray(np.broadcast_to(qb, (P, NBITS)))

    nc = _get_nc(mode)
    in_maps = [
        {"x": np.ascontiguousarray(x[b], dtype=np.float32), "cq": cq}
        for b in range(NCORES)
    ]
    kres = run_bass_kernel_spmd(nc, in_maps, list(range(NCORES)), trace=trace)

    outs = []
    for r in kres.results:
        o = np.asarray(r["out"]).reshape(C, C, VD)
        outs.append(o[np.arange(C), np.arange(C)].sum(axis=0))
    return np.stack(outs).astype(np.float32), kres


def kernel(x, query_addr):
    return run(x, query_addr)[0]


# revision 18
# speedup vs baseline: 1.2088x; 1.0707x over previous
"""BinaryMatchAttention Trainium2 kernel.

reference semantics (per batch b):
    qb[k]   = (query_addr >> k) & 1                 k in [0, 16)
    w[s]    = prod_k (1 - |x[b, s, 96+k] - qb[k]|)
    out[b,d]= sum_s w[s] * x[b, s, d]               d in [0, 96)

Sharding: data-parallel over batch, one NeuronCore per batch element
(B == 8 == n_cores), no collectives.

Per-core plan (x_core [32768, 128] fp32 in HBM):
  - seq is blocked into 64 "superchunks" of 512 positions; partition p
    holds the 4 consecutive rows  s = sc*512 + 4p + r  (r in [0,4)).
    Loading all 128 cols of 4 consecutive rows gives 2 KiB-contiguous
    DMA descriptors (4.5x fewer than a row-per-partition layout, ~23.7
    vs ~19.2 GB/s per SDMA engine measured) at the cost of also reading
    the 16 unused tail columns.
  - match weights: d = bits - qb (DVE), a = |d| (ACT), m = 1 - a (DVE
    2x tensor_scalar), then 4 strided pairwise products (DVE) reduce
    16 -> 1 giving w[p, sc, r].
  - einsum on TensorE: per superchunk,
    psum[4, 384] += w[:, sc, 0:4].T @ v[:, sc, 0:4, 0:96]  (float32r,
    1 cycle/row at N=384), accumulated across all 64 superchunks in one
    PSUM bank. Only the diagonal 96-blocks (r == r') are wanted; the
    host extracts and sums them (24 junk floats per row ignored).
  - float32r (TF32-like reduced-precision PE path) gives ~5e-4 rel err
    on the final output; mode "f32" is an exact-fp32 fallback.
"""

import os
import sys

if "/opt/trn_rl_repo" not in sys.path:
    sys.path.insert(0, "/opt/trn_rl_repo")

import numpy as np

S, D = 32768, 128
VD = 96          # value payload dims
NBITS = 16
BIT0 = 96
P = 128          # partitions
R = 4            # seq rows per partition per superchunk
SC = S // (P * R)   # 64 superchunks
C = R            # chunk-rows fused per matmul (diagonal trick)

# Wave sizes in superchunks (1 superchunk = 512 seq positions). The
# taper at the end shrinks the serial tail after the last DMA lands.
WS = [8] * 7 + [4, 2, 1, 1]
assert sum(WS) == SC

NCORES = 8

# "f32r" : float32r matmuls (1 cycle/row, ~5e-4 rel err)
# "f32"  : plain fp32 matmuls (4 cycles/row, exact)
MM_MODE = os.environ.get("BMA_MM_MODE", "f32r")

_CACHE = {}


def _build(mode):
    import concourse.bacc as bacc
    import concourse.mybir as mybir
    import concourse.tile as tile

    f32 = mybir.dt.float32
    x_dt = mybir.dt.float32r if mode == "f32r" else f32

    nc = bacc.Bacc("TRN2", target_bir_lowering=False, debug=False)
    x = nc.dram_tensor("x", [S, D], x_dt, kind="ExternalInput")
    cq = nc.dram_tensor("cq", [P, NBITS], f32, kind="ExternalInput")
    out = nc.dram_tensor("out", [C, C * VD], f32, kind="ExternalOutput")

    # [128(part), 64(superchunk), 4(row), 128(col)]; (row, col) is a
    # contiguous 2 KiB run in HBM for each (part, superchunk).
    xr = x.ap().rearrange("(sc p r) d -> p sc r d", p=P, r=R)

    last_g = SC - 1

    with tile.TileContext(nc) as tc:
        with (
            tc.tile_pool(name="const", bufs=1) as cpool,
            tc.tile_pool(name="v", bufs=5) as vpool,
            tc.tile_pool(name="wk", bufs=2) as wpool,
            tc.tile_pool(name="ps", bufs=1, space="PSUM") as ppool,
            tc.tile_pool(name="res", bufs=1) as rpool,
        ):
            cqt = cpool.tile([P, 1, 1, NBITS], f32)
            nc.sync.dma_start(
                cqt[:], cq.ap().rearrange("p (a b k) -> p a b k", a=1, b=1)
            )

            acc = ppool.tile([C, C * VD], f32)

            g = 0
            sc0 = 0
            for ib, W in enumerate(WS):
                vt = vpool.tile([P, W, R, D], x_dt, tag="vt")
                # Sync's only job: keep descriptors streaming into the
                # HWDGE ring. ACT stays free for the Abs step.
                nc.sync.dma_start(vt[:], xr[:, sc0 : sc0 + W, :, :])
                sc0 += W

                bits = vt[:, :, :, BIT0 : BIT0 + NBITS]
                if mode == "f32r":
                    bits = bits.bitcast(f32)
                d = wpool.tile([P, W, R, NBITS], f32, tag="d")
                nc.vector.tensor_sub(
                    d[:], bits, cqt[:].broadcast_to([P, W, R, NBITS])
                )
                a = wpool.tile([P, W, R, NBITS], f32, tag="a")
                nc.scalar.activation(a[:], d[:], mybir.ActivationFunctionType.Abs)
                m = wpool.tile([P, W, R, NBITS], f32, tag="m")
                nc.vector.tensor_scalar(
                    m[:], a[:], -1.0, 1.0,
                    op0=mybir.AluOpType.mult, op1=mybir.AluOpType.add,
                )
                p8 = wpool.tile([P, W, R, 8], f32, tag="p8")
                nc.vector.tensor_mul(p8[:], m[:, :, :, 0::2], m[:, :, :, 1::2])
                p4 = wpool.tile([P, W, R, 4], f32, tag="p4")
                nc.vector.tensor_mul(p4[:], p8[:, :, :, 0::2], p8[:, :, :, 1::2])
                p2 = wpool.tile([P, W, R, 2], f32, tag="p2")
                nc.vector.tensor_mul(p2[:], p4[:, :, :, 0::2], p4[:, :, :, 1::2])
                # final tree level writes the weight tile, rounded to the
                # matmul dtype so the verifier sees an f32r producer
                w = wpool.tile([P, W, R, 1], x_dt, tag="w")
                nc.vector.tensor_mul(w[:], p2[:, :, :, 0::2], p2[:, :, :, 1::2])

                for j in range(W):
                    lhsT = w[:, j, 0:R, 0]            # [128, 4]
                    rhs = vt[:, j, 0:R, 0:VD]         # [128, 4, 96]
                    nc.tensor.matmul(
                        acc[:],
                        lhsT,
                        rhs,
                        start=(g == 0),
                        stop=(g == last_g),
                    )
                    g += 1

            res = rpool.tile([C, C * VD], f32)
            nc.vector.tensor_copy(res[:], acc[:])
            nc.sync.dma_start(out.ap(), res[:])

    nc.compile()
    return nc


def _get_nc(mode):
    if mode not in _CACHE:
        _CACHE[mode] = _build(mode)
    return _CACHE[mode]


def run(x, query_addr, trace=False, mode=None):
    """Returns (output [B, 96] float32, BassKernelResults)."""
    from concourse.bass_utils import run_bass_kernel_spmd

    mode = mode or MM_MODE
    x = np.asarray(x)
    qa = int(np.asarray(query_addr))
    assert x.shape == (NCORES, S, D), x.shape

    qb = np.array([(qa >> k) & 1 for k in range(NBITS)], dtype=np.float32)
    cq = np.ascontiguousarray(np.broadcast_to(qb, (P, NBITS)))

    nc = _get_nc(mode)
    in_maps = [
        {"x": np.ascontiguousarray(x[b], dtype=np.float32), "cq": cq}
        for b in range(NCORES)
    ]
    kres = run_bass_kernel_spmd(nc, in_maps, list(range(NCORES)), trace=trace)

    outs = []
    for r in kres.results:
        o = np.asarray(r["out"]).reshape(C, C, VD)
        outs.append(o[np.arange(C), np.arange(C)].sum(axis=0))
    return np.stack(outs).astype(np.float32), kres


def kernel(x, query_addr):
    return run(x, query_addr)[0]
